# revision 12
# baseline (speedup 1.0000x reference)
"""Trainium2 Bass kernel for nn_ArmatureLayer (linear blend skinning).

reference math:
    q  = normalize(pose_bone_quaternions)             # [64, 4]
    T  = sequential FK chain over 64 bones            # [64, 4, 4]  (tiny)
    verts = einsum('bvi,bij->vj', vbi, T)[:, :3]      # [400000, 3] (409.6 MB read)
    returns (verts, T, pose_bone_scales)

Strategy (memory-bound, 8 NeuronCores):
  * FK chain (64 sequential 4x4 ops) on host in numpy -- negligible, and T is
    itself a returned output.
  * Shard the 400k vertices across 8 cores (50k each).  Per core, vbi is
    pre-packed on host into [nchunk, 128, 2, V]: partition p = 2*b + i' holds
    (bone b, quaternion component i = 2*g + i') for contraction group g, with
    V vertices contiguous per (partition, g).  Every input DMA is one fully
    contiguous 5 MB, 128-partition transfer.
  * Device compute: out^T[j, v] = sum_g (T_g[k, j])^T @ X_g[k, v] with
    K = 128 = (64 bones x 2 components) per group -- just TWO fp32r matmuls
    (N=500, full 128-partition contraction) accumulate each [4, 500] PSUM
    tile.  fp32r streams 1 column/cycle at N>=256, so PE work is
    2 cycles/vertex, comfortably under the DMA roofline even at the cold
    1.2 GHz clock (self-loading fp32r matmuls serialize; no row-tiling
    overlap exists for them).
  * PSUM -> SBUF copies alternate between the vector and scalar engines,
    then one DMA per chunk writes out^T [4, 5000] to DRAM.
  * Host transposes the gathered [4, 400000] to [400000, 4] and slices :3.

The BIR is post-processed by _legalize_waits: this walrus encodes at most
one sync-wait per ISA instruction, so extra waits are hoisted onto
EventSemaphore instructions inserted before, on the same engine.
"""

import json

import numpy as np

import concourse.bass as bass
import concourse.mybir as mybir
from concourse import tile
from concourse.bass_utils import run_bass_kernel_spmd

F32 = mybir.dt.float32
F32R = mybir.dt.float32r

NB = 64            # bones
NV = 400_000       # vertices
NCORES = 8
NV_CORE = NV // NCORES   # 50000
CHUNK = 2500             # vertices per chunk (per-partition line = 20KB)
NCHUNK = NV_CORE // CHUNK  # 20
NSUB = CHUNK // 500      # matmul N=500 groups per chunk


def _legalize_waits(nc):
    """The walrus on this stack encodes at most ONE sync-wait per ISA
    instruction, but Tile packs several onto matmuls/drains/DMAs.  Split:
    keep one wait on the instruction and hoist the rest onto EventSemaphore
    instructions inserted just before it on the same engine (per-engine
    program order makes this semantics-preserving).  Installed by wrapping
    nc.to_json_bytes, so every compile path sees the legalized BIR."""
    orig = nc.to_json_bytes

    def patched():
        m = json.loads(orig())
        for fn in m["functions"]:
            for blk in fn["blocks"]:
                out = []
                for inst in blk.get("instructions", []):
                    si = inst.get("sync_info") or {}
                    waits = si.get("on_wait") or []
                    if len(waits) > 1:
                        for k, w in enumerate(waits[:-1]):
                            out.append({
                                "debug": inst.get("debug", 0),
                                "engine": inst["engine"],
                                "ins": [],
                                "name": f"{inst['name']}_hw{k}",
                                "opcode": "EventSemaphore",
                                "outs": [],
                                "sync_info": {"on_update": [], "on_wait": [w]},
                            })
                        si["on_wait"] = [waits[-1]]
                    out.append(inst)
                blk["instructions"] = out
        return json.dumps(m).encode()

    nc.to_json_bytes = patched
    return nc


def build_graph(nchunk=NCHUNK, chunk=CHUNK):
    nsub = chunk // 500
    assert chunk % 500 == 0
    nv = nchunk * chunk
    nc = bass.Bass()
    # float32r end-to-end for the matmul operands: same 4-byte storage as
    # f32, but the BIR verifier requires the f32r matmult inputs to be typed
    # (and thus "rounded") f32r along the whole chain.
    vbi = nc.declare_dram_parameter("vbi", [nchunk, 128, 2, chunk], F32R, isOutput=False)
    tmat = nc.declare_dram_parameter("tmat", [128, 2, 4], F32R, isOutput=False)
    out_t = nc.declare_dram_parameter("out_t", [4, nv], F32, isOutput=True)

    with tile.TileContext(nc) as tc:
        with (
            tc.tile_pool(name="tpool", bufs=1) as tpool,
            tc.tile_pool(name="xpool", bufs=5) as xpool,
            tc.tile_pool(name="opool", bufs=4) as opool,
            tc.tile_pool(name="pspool", bufs=6, space="PSUM") as pspool,
        ):
            tsb = tpool.tile([128, 2, 4], F32R)
            nc.sync.dma_start(tsb[:], tmat[:])
            for c in range(nchunk):
                x = xpool.tile([128, 2, chunk], F32R, tag="x")
                nc.sync.dma_start(x[:], vbi[c])
                o = opool.tile([4, chunk], F32, tag="o")
                for vs in range(nsub):
                    ps = pspool.tile([4, 500], F32, tag="ps")
                    for g in range(2):
                        nc.tensor.matmul(
                            ps[:],
                            tsb[:, g, :],
                            x[:, g, vs * 500:(vs + 1) * 500],
                            start=(g == 0),
                            stop=(g == 1),
                        )
                    dst = o[:, vs * 500:(vs + 1) * 500]
                    if vs % 2 == 0:
                        nc.vector.tensor_copy(dst, ps[:])
                    else:
                        nc.scalar.copy(dst, ps[:])
                nc.gpsimd.dma_start(out_t[:, c * chunk:(c + 1) * chunk], o[:])
    return _legalize_waits(nc)


def _fk(pose_bone_quaternions, relative_pose_bone_translations, offset, bone_parents):
    """Mirror of the reference FK chain (incl. the preserved bug: the
    UNSCALED transform feeds the children, so scales are dead)."""
    q = np.asarray(pose_bone_quaternions, np.float32)
    rel_t = np.asarray(relative_pose_bone_translations, np.float32)
    off = np.asarray(offset, np.float32)
    parents = np.asarray(bone_parents).astype(np.int64)
    nb = q.shape[0]
    q = q / np.linalg.norm(q, axis=1, keepdims=True)
    w, x, y, z = q[:, 0], q[:, 1], q[:, 2], q[:, 3]
    # R rows per reference; we store Rt = R.T directly.
    R = np.empty((nb, 3, 3), np.float32)
    R[:, 0, 0] = 1 - 2 * (y * y + z * z)
    R[:, 0, 1] = 2 * (x * y - w * z)
    R[:, 0, 2] = 2 * (x * z + w * y)
    R[:, 1, 0] = 2 * (x * y + w * z)
    R[:, 1, 1] = 1 - 2 * (x * x + z * z)
    R[:, 1, 2] = 2 * (y * z - w * x)
    R[:, 2, 0] = 2 * (x * z - w * y)
    R[:, 2, 1] = 2 * (y * z + w * x)
    R[:, 2, 2] = 1 - 2 * (x * x + y * y)
    Rt = R.transpose(0, 2, 1)
    T = np.zeros((nb, 4, 4), np.float32)
    for i in range(nb):
        if parents[i] == -1:
            t4 = np.concatenate([rel_t[i, :3] + off, np.ones((1,), np.float32)])
        else:
            t4 = rel_t[i] @ T[parents[i]]
        T[i, :3, :3] = Rt[i]
        T[i, 3, :] = t4
    return T


def _pack_vbi(vbi):
    """[64, 400000, 4] -> [8, nchunk, 128, 2, CHUNK] with partition
    p = 2*b + i' and groups g over i = 2*g + i'."""
    v = np.ascontiguousarray(np.asarray(vbi, np.float32))
    # one strided pass: [b, v, i] -> [b, i, v] (the only non-block transpose)
    vt = np.ascontiguousarray(v.transpose(0, 2, 1))        # [64, 4, 400000]
    vt = vt.reshape(NB, 2, 2, NCORES, NCHUNK, CHUNK)       # b, g, i', core, c, v
    vt = vt.transpose(3, 4, 0, 2, 1, 5)                    # core, c, b, i', g, v
    return np.ascontiguousarray(vt).reshape(NCORES, NCHUNK, 128, 2, CHUNK)


def _pack_tmat(T):
    """[64, 4, 4] -> [128, 2, 4]: tmat[2*b + i', g, j] = T[b, 2*g + i', j]."""
    tm = T.reshape(NB, 2, 2, 4).transpose(0, 2, 1, 3)      # b, i', g, j
    return np.ascontiguousarray(tm).reshape(128, 2, 4)


_GRAPH = None


def _graph():
    global _GRAPH
    if _GRAPH is None:
        _GRAPH = build_graph()
    return _GRAPH


def _run(inputs, trace=False):
    T = _fk(
        inputs["pose_bone_quaternions"],
        inputs["relative_pose_bone_translations"],
        inputs["offset"],
        inputs["bone_parents"],
    )
    packed = _pack_vbi(inputs["vertices_bone_inverted"])
    tmat = _pack_tmat(T)
    in_maps = [{"vbi": packed[i], "tmat": tmat} for i in range(NCORES)]
    res = run_bass_kernel_spmd(
        _graph(), in_maps, core_ids=list(range(NCORES)), trace=trace
    )
    out = np.concatenate([res.results[i]["out_t"] for i in range(NCORES)], axis=1)
    verts = np.ascontiguousarray(out.T[:, :3])
    scales = np.ascontiguousarray(np.asarray(inputs["pose_bone_scales"], np.float32))
    return (verts, T, scales), res


def kernel(**inputs):
    out, _ = _run(inputs, trace=False)
    return out


def kernel_traced(**inputs):
    return _run(inputs, trace=True)


# revision 14
# speedup vs baseline: 1.1555x; 1.1555x over previous
"""Trainium2 Bass kernel for nn_ArmatureLayer (linear blend skinning).

reference math:
    q  = normalize(pose_bone_quaternions)             # [64, 4]
    T  = sequential FK chain over 64 bones            # [64, 4, 4]  (tiny)
    verts = einsum('bvi,bij->vj', vbi, T)[:, :3]      # [400000, 3] (409.6 MB read)
    returns (verts, T, pose_bone_scales)

Strategy (memory-bound, 8 NeuronCores):
  * FK chain (64 sequential 4x4 ops) on host in numpy -- negligible, and T is
    itself a returned output.
  * Shard the 400k vertices across 8 cores (50k each).  Per core, vbi is
    pre-packed on host into [nchunk, 128, 2, V]: partition p = 2*b + i' holds
    (bone b, quaternion component i = 2*g + i') for contraction group g, with
    V vertices contiguous per (partition, g).  Every input DMA is one fully
    contiguous 5 MB, 128-partition transfer.
  * Device compute: out^T[j, v] = sum_g (T_g[k, j])^T @ X_g[k, v] with
    K = 128 = (64 bones x 2 components) per group -- just TWO fp32r matmuls
    (N=500, full 128-partition contraction) accumulate each [4, 500] PSUM
    tile.  fp32r streams 1 column/cycle at N>=256, so PE work is
    2 cycles/vertex, comfortably under the DMA roofline even at the cold
    1.2 GHz clock (self-loading fp32r matmuls serialize; no row-tiling
    overlap exists for them).
  * PSUM -> SBUF copies alternate between the vector and scalar engines,
    then one DMA per chunk writes out^T [4, 5000] to DRAM.
  * Host transposes the gathered [4, 400000] to [400000, 4] and slices :3.

The BIR is post-processed by _legalize_waits: this walrus encodes at most
one sync-wait per ISA instruction, so extra waits are hoisted onto
EventSemaphore instructions inserted before, on the same engine.
"""

import json

import numpy as np

import concourse.bass as bass
import concourse.mybir as mybir
from concourse import tile
from concourse.bass_utils import run_bass_kernel_spmd

F32 = mybir.dt.float32
F32R = mybir.dt.float32r

NB = 64            # bones
NV = 400_000       # vertices
NCORES = 8
NV_CORE = NV // NCORES   # 50000
CHUNK = 2500             # vertices per chunk (per-partition line = 20KB)
NCHUNK = NV_CORE // CHUNK  # 20
NSUB = CHUNK // 500      # matmul N=500 groups per chunk


def _legalize_waits(nc):
    """The walrus on this stack encodes at most ONE sync-wait per ISA
    instruction, but Tile packs several onto matmuls/drains/DMAs.  Split:
    keep one wait on the instruction and hoist the rest onto EventSemaphore
    instructions inserted just before it on the same engine (per-engine
    program order makes this semantics-preserving).  Installed by wrapping
    nc.to_json_bytes, so every compile path sees the legalized BIR."""
    orig = nc.to_json_bytes

    def patched():
        m = json.loads(orig())
        for fn in m["functions"]:
            for blk in fn["blocks"]:
                out = []
                for inst in blk.get("instructions", []):
                    si = inst.get("sync_info") or {}
                    waits = si.get("on_wait") or []
                    if len(waits) > 1:
                        for k, w in enumerate(waits[:-1]):
                            out.append({
                                "debug": inst.get("debug", 0),
                                "engine": inst["engine"],
                                "ins": [],
                                "name": f"{inst['name']}_hw{k}",
                                "opcode": "EventSemaphore",
                                "outs": [],
                                "sync_info": {"on_update": [], "on_wait": [w]},
                            })
                        si["on_wait"] = [waits[-1]]
                    out.append(inst)
                blk["instructions"] = out
        return json.dumps(m).encode()

    nc.to_json_bytes = patched
    return nc


def build_graph(nchunk=NCHUNK, chunk=CHUNK):
    nsub = chunk // 500
    assert chunk % 500 == 0
    nv = nchunk * chunk
    nc = bass.Bass()
    # float32r end-to-end for the matmul operands: same 4-byte storage as
    # f32, but the BIR verifier requires the f32r matmult inputs to be typed
    # (and thus "rounded") f32r along the whole chain.
    vbi = nc.declare_dram_parameter("vbi", [nchunk, 128, 2, chunk], F32R, isOutput=False)
    tmat = nc.declare_dram_parameter("tmat", [128, 2, 4], F32R, isOutput=False)
    out_t = nc.declare_dram_parameter("out_t", [4, nv], F32, isOutput=True)

    with tile.TileContext(nc) as tc:
        with (
            tc.tile_pool(name="tpool", bufs=1) as tpool,
            tc.tile_pool(name="xpool", bufs=5) as xpool,
            tc.tile_pool(name="opool", bufs=4) as opool,
            tc.tile_pool(name="pspool", bufs=6, space="PSUM") as pspool,
        ):
            tsb = tpool.tile([128, 2, 4], F32R)
            nc.sync.dma_start(tsb[:], tmat[:])
            for c in range(nchunk):
                last = c == nchunk - 1
                o = opool.tile([4, chunk], F32, tag="o")
                if not last:
                    x = xpool.tile([128, 2, chunk], F32R, tag="x")
                    nc.sync.dma_start(x[:], vbi[c])
                for vs in range(nsub):
                    if last:
                        # stream the final chunk in 500-vertex pieces so the
                        # tail compute + output drain starts as soon as the
                        # first piece lands instead of after the full chunk
                        xt = xpool.tile([128, 2, 500], F32R, tag="xt")
                        nc.sync.dma_start(xt[:], vbi[c][:, :, vs * 500:(vs + 1) * 500])
                    ps = pspool.tile([4, 500], F32, tag="ps")
                    for g in range(2):
                        rhs = xt[:, g, :] if last else x[:, g, vs * 500:(vs + 1) * 500]
                        nc.tensor.matmul(
                            ps[:],
                            tsb[:, g, :],
                            rhs,
                            start=(g == 0),
                            stop=(g == 1),
                        )
                    dst = o[:, vs * 500:(vs + 1) * 500]
                    if vs % 2 == 0:
                        nc.vector.tensor_copy(dst, ps[:])
                    else:
                        nc.scalar.copy(dst, ps[:])
                    if last:
                        nc.gpsimd.dma_start(
                            out_t[:, c * chunk + vs * 500: c * chunk + (vs + 1) * 500],
                            dst,
                        )
                if not last:
                    nc.gpsimd.dma_start(out_t[:, c * chunk:(c + 1) * chunk], o[:])
    return _legalize_waits(nc)


def _fk(pose_bone_quaternions, relative_pose_bone_translations, offset, bone_parents):
    """Mirror of the reference FK chain (incl. the preserved bug: the
    UNSCALED transform feeds the children, so scales are dead)."""
    q = np.asarray(pose_bone_quaternions, np.float32)
    rel_t = np.asarray(relative_pose_bone_translations, np.float32)
    off = np.asarray(offset, np.float32)
    parents = np.asarray(bone_parents).astype(np.int64)
    nb = q.shape[0]
    q = q / np.linalg.norm(q, axis=1, keepdims=True)
    w, x, y, z = q[:, 0], q[:, 1], q[:, 2], q[:, 3]
    # R rows per reference; we store Rt = R.T directly.
    R = np.empty((nb, 3, 3), np.float32)
    R[:, 0, 0] = 1 - 2 * (y * y + z * z)
    R[:, 0, 1] = 2 * (x * y - w * z)
    R[:, 0, 2] = 2 * (x * z + w * y)
    R[:, 1, 0] = 2 * (x * y + w * z)
    R[:, 1, 1] = 1 - 2 * (x * x + z * z)
    R[:, 1, 2] = 2 * (y * z - w * x)
    R[:, 2, 0] = 2 * (x * z - w * y)
    R[:, 2, 1] = 2 * (y * z + w * x)
    R[:, 2, 2] = 1 - 2 * (x * x + y * y)
    Rt = R.transpose(0, 2, 1)
    T = np.zeros((nb, 4, 4), np.float32)
    for i in range(nb):
        if parents[i] == -1:
            t4 = np.concatenate([rel_t[i, :3] + off, np.ones((1,), np.float32)])
        else:
            t4 = rel_t[i] @ T[parents[i]]
        T[i, :3, :3] = Rt[i]
        T[i, 3, :] = t4
    return T


def _pack_vbi(vbi):
    """[64, 400000, 4] -> [8, nchunk, 128, 2, CHUNK] with partition
    p = 2*b + i' and groups g over i = 2*g + i'."""
    v = np.ascontiguousarray(np.asarray(vbi, np.float32))
    # one strided pass: [b, v, i] -> [b, i, v] (the only non-block transpose)
    vt = np.ascontiguousarray(v.transpose(0, 2, 1))        # [64, 4, 400000]
    vt = vt.reshape(NB, 2, 2, NCORES, NCHUNK, CHUNK)       # b, g, i', core, c, v
    vt = vt.transpose(3, 4, 0, 2, 1, 5)                    # core, c, b, i', g, v
    return np.ascontiguousarray(vt).reshape(NCORES, NCHUNK, 128, 2, CHUNK)


def _pack_tmat(T):
    """[64, 4, 4] -> [128, 2, 4]: tmat[2*b + i', g, j] = T[b, 2*g + i', j]."""
    tm = T.reshape(NB, 2, 2, 4).transpose(0, 2, 1, 3)      # b, i', g, j
    return np.ascontiguousarray(tm).reshape(128, 2, 4)


_GRAPH = None


def _graph():
    global _GRAPH
    if _GRAPH is None:
        _GRAPH = build_graph()
    return _GRAPH


def _run(inputs, trace=False):
    T = _fk(
        inputs["pose_bone_quaternions"],
        inputs["relative_pose_bone_translations"],
        inputs["offset"],
        inputs["bone_parents"],
    )
    packed = _pack_vbi(inputs["vertices_bone_inverted"])
    tmat = _pack_tmat(T)
    in_maps = [{"vbi": packed[i], "tmat": tmat} for i in range(NCORES)]
    res = run_bass_kernel_spmd(
        _graph(), in_maps, core_ids=list(range(NCORES)), trace=trace
    )
    out = np.concatenate([res.results[i]["out_t"] for i in range(NCORES)], axis=1)
    verts = np.ascontiguousarray(out.T[:, :3])
    scales = np.ascontiguousarray(np.asarray(inputs["pose_bone_scales"], np.float32))
    return (verts, T, scales), res


def kernel(**inputs):
    out, _ = _run(inputs, trace=False)
    return out


def kernel_traced(**inputs):
    return _run(inputs, trace=True)


# revision 15
# speedup vs baseline: 1.2060x; 1.0437x over previous
"""Trainium2 Bass kernel for nn_ArmatureLayer (linear blend skinning).

reference math:
    q  = normalize(pose_bone_quaternions)             # [64, 4]
    T  = sequential FK chain over 64 bones            # [64, 4, 4]  (tiny)
    verts = einsum('bvi,bij->vj', vbi, T)[:, :3]      # [400000, 3] (409.6 MB read)
    returns (verts, T, pose_bone_scales)

Strategy (memory-bound, 8 NeuronCores):
  * FK chain (64 sequential 4x4 ops) on host in numpy -- negligible, and T is
    itself a returned output.
  * Shard the 400k vertices across 8 cores (50k each).  Per core, vbi is
    pre-packed on host into [nchunk, 128, 2, V]: partition p = 2*b + i' holds
    (bone b, quaternion component i = 2*g + i') for contraction group g, with
    V vertices contiguous per (partition, g).  Every input DMA is one fully
    contiguous 5 MB, 128-partition transfer.
  * Device compute: out^T[j, v] = sum_g (T_g[k, j])^T @ X_g[k, v] with
    K = 128 = (64 bones x 2 components) per group -- just TWO fp32r matmuls
    (N=500, full 128-partition contraction) accumulate each [4, 500] PSUM
    tile.  fp32r streams 1 column/cycle at N>=256, so PE work is
    2 cycles/vertex, comfortably under the DMA roofline even at the cold
    1.2 GHz clock (self-loading fp32r matmuls serialize; no row-tiling
    overlap exists for them).
  * PSUM -> SBUF copies alternate between the vector and scalar engines,
    then one DMA per chunk writes out^T [4, 5000] to DRAM.
  * Host transposes the gathered [4, 400000] to [400000, 4] and slices :3.

The BIR is post-processed by _legalize_waits: this walrus encodes at most
one sync-wait per ISA instruction, so extra waits are hoisted onto
EventSemaphore instructions inserted before, on the same engine.
"""

import json

import numpy as np

import concourse.bass as bass
import concourse.mybir as mybir
from concourse import tile
from concourse.bass_utils import run_bass_kernel_spmd

F32 = mybir.dt.float32
F32R = mybir.dt.float32r

NB = 64            # bones
NV = 400_000       # vertices
NCORES = 8
NV_CORE = NV // NCORES   # 50000
CHUNK = 2500             # vertices per chunk (per-partition line = 20KB)
NCHUNK = NV_CORE // CHUNK  # 20
NSUB = CHUNK // 500      # matmul N=500 groups per chunk


def _legalize_waits(nc):
    """The walrus on this stack encodes at most ONE sync-wait per ISA
    instruction, but Tile packs several onto matmuls/drains/DMAs.  Split:
    keep one wait on the instruction and hoist the rest onto EventSemaphore
    instructions inserted just before it on the same engine (per-engine
    program order makes this semantics-preserving).  Installed by wrapping
    nc.to_json_bytes, so every compile path sees the legalized BIR."""
    orig = nc.to_json_bytes

    def patched():
        m = json.loads(orig())
        for fn in m["functions"]:
            for blk in fn["blocks"]:
                out = []
                for inst in blk.get("instructions", []):
                    si = inst.get("sync_info") or {}
                    waits = si.get("on_wait") or []
                    if len(waits) > 1:
                        for k, w in enumerate(waits[:-1]):
                            out.append({
                                "debug": inst.get("debug", 0),
                                "engine": inst["engine"],
                                "ins": [],
                                "name": f"{inst['name']}_hw{k}",
                                "opcode": "EventSemaphore",
                                "outs": [],
                                "sync_info": {"on_update": [], "on_wait": [w]},
                            })
                        si["on_wait"] = [waits[-1]]
                    out.append(inst)
                blk["instructions"] = out
        return json.dumps(m).encode()

    nc.to_json_bytes = patched
    return nc


def build_graph(nchunk=NCHUNK, chunk=CHUNK):
    nsub = chunk // 500
    assert chunk % 500 == 0
    nv = nchunk * chunk
    nc = bass.Bass()
    # float32r end-to-end for the matmul operands: same 4-byte storage as
    # f32, but the BIR verifier requires the f32r matmult inputs to be typed
    # (and thus "rounded") f32r along the whole chain.
    vbi = nc.declare_dram_parameter("vbi", [nchunk, 128, 2, chunk], F32R, isOutput=False)
    tmat = nc.declare_dram_parameter("tmat", [128, 2, 4], F32R, isOutput=False)
    out_t = nc.declare_dram_parameter("out_t", [4, nv], F32, isOutput=True)

    with tile.TileContext(nc) as tc:
        with (
            tc.tile_pool(name="tpool", bufs=1) as tpool,
            tc.tile_pool(name="xpool", bufs=5) as xpool,
            tc.tile_pool(name="opool", bufs=4) as opool,
            tc.tile_pool(name="pspool", bufs=6, space="PSUM") as pspool,
        ):
            tsb = tpool.tile([128, 2, 4], F32R)
            # tmat rides the ACT HWDGE ring so its first-DMA latency overlaps
            # the first big chunk DMA on the SP ring
            nc.scalar.dma_start(tsb[:], tmat[:])
            for c in range(nchunk):
                last = c == nchunk - 1
                o = opool.tile([4, chunk], F32, tag="o")
                if not last:
                    x = xpool.tile([128, 2, chunk], F32R, tag="x")
                    nc.sync.dma_start(x[:], vbi[c])
                for vs in range(nsub):
                    if last:
                        # stream the final chunk in 500-vertex pieces so the
                        # tail compute + output drain starts as soon as the
                        # first piece lands instead of after the full chunk
                        xt = xpool.tile([128, 2, 500], F32R, tag="xt")
                        nc.sync.dma_start(xt[:], vbi[c][:, :, vs * 500:(vs + 1) * 500])
                    ps = pspool.tile([4, 500], F32, tag="ps")
                    for g in range(2):
                        rhs = xt[:, g, :] if last else x[:, g, vs * 500:(vs + 1) * 500]
                        nc.tensor.matmul(
                            ps[:],
                            tsb[:, g, :],
                            rhs,
                            start=(g == 0),
                            stop=(g == 1),
                        )
                    dst = o[:, vs * 500:(vs + 1) * 500]
                    if vs % 2 == 0:
                        nc.vector.tensor_copy(dst, ps[:])
                    else:
                        nc.scalar.copy(dst, ps[:])
                    if last:
                        nc.gpsimd.dma_start(
                            out_t[:, c * chunk + vs * 500: c * chunk + (vs + 1) * 500],
                            dst,
                        )
                if not last:
                    nc.gpsimd.dma_start(out_t[:, c * chunk:(c + 1) * chunk], o[:])
    return _legalize_waits(nc)


def _fk(pose_bone_quaternions, relative_pose_bone_translations, offset, bone_parents):
    """Mirror of the reference FK chain (incl. the preserved bug: the
    UNSCALED transform feeds the children, so scales are dead)."""
    q = np.asarray(pose_bone_quaternions, np.float32)
    rel_t = np.asarray(relative_pose_bone_translations, np.float32)
    off = np.asarray(offset, np.float32)
    parents = np.asarray(bone_parents).astype(np.int64)
    nb = q.shape[0]
    q = q / np.linalg.norm(q, axis=1, keepdims=True)
    w, x, y, z = q[:, 0], q[:, 1], q[:, 2], q[:, 3]
    # R rows per reference; we store Rt = R.T directly.
    R = np.empty((nb, 3, 3), np.float32)
    R[:, 0, 0] = 1 - 2 * (y * y + z * z)
    R[:, 0, 1] = 2 * (x * y - w * z)
    R[:, 0, 2] = 2 * (x * z + w * y)
    R[:, 1, 0] = 2 * (x * y + w * z)
    R[:, 1, 1] = 1 - 2 * (x * x + z * z)
    R[:, 1, 2] = 2 * (y * z - w * x)
    R[:, 2, 0] = 2 * (x * z - w * y)
    R[:, 2, 1] = 2 * (y * z + w * x)
    R[:, 2, 2] = 1 - 2 * (x * x + y * y)
    Rt = R.transpose(0, 2, 1)
    T = np.zeros((nb, 4, 4), np.float32)
    for i in range(nb):
        if parents[i] == -1:
            t4 = np.concatenate([rel_t[i, :3] + off, np.ones((1,), np.float32)])
        else:
            t4 = rel_t[i] @ T[parents[i]]
        T[i, :3, :3] = Rt[i]
        T[i, 3, :] = t4
    return T


def _pack_vbi(vbi):
    """[64, 400000, 4] -> [8, nchunk, 128, 2, CHUNK] with partition
    p = 2*b + i' and groups g over i = 2*g + i'."""
    v = np.ascontiguousarray(np.asarray(vbi, np.float32))
    # one strided pass: [b, v, i] -> [b, i, v] (the only non-block transpose)
    vt = np.ascontiguousarray(v.transpose(0, 2, 1))        # [64, 4, 400000]
    vt = vt.reshape(NB, 2, 2, NCORES, NCHUNK, CHUNK)       # b, g, i', core, c, v
    vt = vt.transpose(3, 4, 0, 2, 1, 5)                    # core, c, b, i', g, v
    return np.ascontiguousarray(vt).reshape(NCORES, NCHUNK, 128, 2, CHUNK)


def _pack_tmat(T):
    """[64, 4, 4] -> [128, 2, 4]: tmat[2*b + i', g, j] = T[b, 2*g + i', j]."""
    tm = T.reshape(NB, 2, 2, 4).transpose(0, 2, 1, 3)      # b, i', g, j
    return np.ascontiguousarray(tm).reshape(128, 2, 4)


_GRAPH = None


def _graph():
    global _GRAPH
    if _GRAPH is None:
        _GRAPH = build_graph()
    return _GRAPH


def _run(inputs, trace=False):
    T = _fk(
        inputs["pose_bone_quaternions"],
        inputs["relative_pose_bone_translations"],
        inputs["offset"],
        inputs["bone_parents"],
    )
    packed = _pack_vbi(inputs["vertices_bone_inverted"])
    tmat = _pack_tmat(T)
    in_maps = [{"vbi": packed[i], "tmat": tmat} for i in range(NCORES)]
    res = run_bass_kernel_spmd(
        _graph(), in_maps, core_ids=list(range(NCORES)), trace=trace
    )
    out = np.concatenate([res.results[i]["out_t"] for i in range(NCORES)], axis=1)
    verts = np.ascontiguousarray(out.T[:, :3])
    scales = np.ascontiguousarray(np.asarray(inputs["pose_bone_scales"], np.float32))
    return (verts, T, scales), res


def kernel(**inputs):
    out, _ = _run(inputs, trace=False)
    return out


def kernel_traced(**inputs):
    return _run(inputs, trace=True)
